# revision 7
# baseline (speedup 1.0000x reference)
"""GPT2 attention (B=4, S=2048, D=1024, H=16) on 8 trn2 cores.

Sharding: data-parallel over batch (4) x tensor-parallel over heads (2 groups
of 8). Core c handles batch c//2, head group c%2. Each core computes its
partial output projection (row-split c_proj); the host sums the two partials
per batch and adds the (host-folded) biases.

Per-core kernel (all matmuls in float32r ~ tf32 precision, fp32 accumulate):
  B: x^T via PE transposes -> QKV projections (q^T,k^T feature-major; v
     natural with a ones column appended for softmax row sums)
  C: causal attention, transposed scores: scoresT[sk,sq] = k^T.T @ q^T,
     p^T = exp(scoresT/8) (no max-subtract; |scores| is small), diagonal
     blocks masked by an upper-triangular 0/1 multiply, attn^T accumulated
     in PSUM over sk blocks via lhsT=[v|1]; normalization by the ones-column
     row sum (reciprocal + K=1 broadcast matmul).
  D: out_partial = attnT.T @ c_proj_w (row slice).
"""
import sys

sys.path.insert(0, "/opt/trn_rl_repo")

import numpy as np
from contextlib import ExitStack

import concourse.bass as bass
import concourse.bacc as bacc
import concourse.mybir as mybir
import concourse.tile as tile
from concourse.masks import make_identity, make_upper_triangular
from concourse.bass_utils import run_bass_kernel_spmd

F32 = mybir.dt.float32
F32R = mybir.dt.float32r
AF = mybir.ActivationFunctionType
OP = mybir.AluOpType

B, S, D, H = 4, 2048, 1024, 16
DH = 64            # head dim
NCORES = 8
GH = 8             # heads per core
GD = GH * DH       # 512 feature cols per core
ST = S // 128      # 16 s-tiles
KB = D // 128      # 8 contraction blocks
NJ = S // 512      # 4 sq chunks
MT = GD // 128     # 4 m-tiles (= head pairs)


def build_module(repeat=1):
    nc = bacc.Bacc(None, target_bir_lowering=False, debug=False)

    x = nc.declare_dram_parameter("x", [S, D], F32, isOutput=False)
    wq = nc.declare_dram_parameter("wq", [D, GD], F32, isOutput=False)
    wk = nc.declare_dram_parameter("wk", [D, GD], F32, isOutput=False)
    wv = nc.declare_dram_parameter("wv", [D, GD], F32, isOutput=False)
    wp = nc.declare_dram_parameter("wp", [GD, D], F32, isOutput=False)
    bqk = nc.declare_dram_parameter("bqk", [128, 2 * MT], F32, isOutput=False)
    out = nc.declare_dram_parameter("out", [S, D], F32, isOutput=True)

    with tile.TileContext(nc) as tc, ExitStack() as rctx:
        if repeat > 1:
            rctx.enter_context(tc.For_i(0, repeat, 1))
        _build_body(nc, tc, x, wq, wk, wv, wp, bqk, out)
    nc.compile()
    return nc


def _build_body(nc, tc, x, wq, wk, wv, wp, bqk, out):
    with ExitStack() as ctx:
        const = ctx.enter_context(tc.tile_pool(name="const", bufs=1))
        wpool = ctx.enter_context(tc.tile_pool(name="wpool", bufs=1))
        wppool = ctx.enter_context(tc.tile_pool(name="wppool", bufs=1))
        resid = ctx.enter_context(tc.tile_pool(name="resid", bufs=1))
        dram = ctx.enter_context(tc.tile_pool(name="dram", bufs=1, space="DRAM"))

        # ---- constants ----
        ident = const.tile([128, 128], F32)
        make_identity(nc, ident[:])
        tri = const.tile([128, 128], F32)  # 1 where col >= row else 0
        make_upper_triangular(nc, tri[:], val=1.0, diag=True)
        ones64_f = const.tile([1, 64], F32)
        nc.gpsimd.memset(ones64_f[:], 1.0)
        ones64 = const.tile([1, 64], F32R)
        nc.vector.tensor_copy(ones64[:], ones64_f[:])
        ones_v = const.tile([128, ST * GH], F32)
        nc.gpsimd.memset(ones_v[:], 1.0)
        bqk_sb = const.tile([128, 2 * MT], F32)
        nc.sync.dma_start(bqk_sb[:], bqk.ap())

        # ---- weights: load f32, convert to f32r ----
        wq_sb = [wpool.tile([128, GD], F32R, name=f"wq{k}") for k in range(KB)]
        wk_sb = [wpool.tile([128, GD], F32R, name=f"wk{k}") for k in range(KB)]
        wv_sb = [wpool.tile([128, GD], F32R, name=f"wv{k}") for k in range(KB)]
        wp_sb = [wppool.tile([128, 512], F32R, name=f"wp{i}") for i in range(8)]
        with tc.tile_pool(name="wstage", bufs=3) as wstage:
            for k in range(KB):
                for name, dr, sb in (("q", wq, wq_sb), ("k", wk, wk_sb),
                                     ("v", wv, wv_sb)):
                    st_ = wstage.tile([128, GD], F32, name="wst")
                    nc.sync.dma_start(st_[:], dr.ap()[k * 128:(k + 1) * 128, :])
                    nc.vector.tensor_copy(sb[k][:], st_[:])
            for k4 in range(4):
                for n in range(2):
                    st_ = wstage.tile([128, 512], F32, name="wst2")
                    nc.sync.dma_start(
                        st_[:], wp.ap()[k4 * 128:(k4 + 1) * 128,
                                        n * 512:(n + 1) * 512])
                    nc.vector.tensor_copy(wp_sb[k4 * 2 + n][:], st_[:])

        # ---- residents ----
        kT_sb = [resid.tile([128, S], F32R, name=f"kT{m}") for m in range(MT)]
        # v with ones column: [128 part = s-within-block, block i, head, 65]
        v_sb = resid.tile([128, ST, GH, DH + 1], F32R)
        nc.vector.tensor_copy(
            v_sb[:, :, :, DH],
            ones_v[:].rearrange("p (a b) -> p a b", a=ST))

        qT_dr = [dram.tile([128, S], F32R, name=f"qTd{m}") for m in range(MT)]
        aT_dr = [dram.tile([128, S], F32R, name=f"aTd{m}") for m in range(MT)]

        # ================= phase B: transposes + QKV =================
        with tc.tile_pool(name="xin", bufs=5) as xin, \
             tc.tile_pool(name="xTp", bufs=2) as xTp, \
             tc.tile_pool(name="qst", bufs=3) as qst, \
             tc.tile_pool(name="ptr", bufs=2, space="PSUM") as ptr, \
             tc.tile_pool(name="pmm", bufs=3, space="PSUM") as pmm:
            for j in range(NJ):
                xts = []
                for st_i in range(4):
                    xt = xin.tile([128, D], F32, name="xt")
                    nc.sync.dma_start(
                        xt[:], x.ap()[(4 * j + st_i) * 128:
                                      (4 * j + st_i + 1) * 128, :])
                    xts.append(xt)
                xT = xTp.tile([128, KB, 512], F32R, name="xT")
                for k in range(KB):
                    pt = ptr.tile([128, 4, 128], F32, name="pt")
                    for st_i in range(4):
                        nc.tensor.matmul(
                            pt[:, st_i, :],
                            lhsT=xts[st_i][:, k * 128:(k + 1) * 128],
                            rhs=ident[:], is_transpose=True,
                            start=(st_i == 0), stop=(st_i == 3))
                    nc.vector.tensor_copy(xT[:, k, :], pt[:, :, :])

                # q^T, k^T : out[d', s-chunk]
                for m in range(MT):
                    for which, wsb, bcol in ((0, wq_sb, m), (1, wk_sb, MT + m)):
                        ps = pmm.tile([128, 512], F32, name="ps")
                        for k in range(KB):
                            nc.tensor.matmul(
                                ps[:], lhsT=wsb[k][:, m * 128:(m + 1) * 128],
                                rhs=xT[:, k, :],
                                start=(k == 0), stop=(k == KB - 1))
                        if which == 0:
                            qs = qst.tile([128, 512], F32R, name="qs")
                            nc.scalar.activation(
                                qs[:], ps[:], AF.Identity,
                                bias=bqk_sb[:, bcol:bcol + 1])
                            nc.sync.dma_start(
                                qT_dr[m][:, j * 512:(j + 1) * 512], qs[:])
                        else:
                            nc.scalar.activation(
                                kT_sb[m][:, j * 512:(j + 1) * 512], ps[:],
                                AF.Identity, bias=bqk_sb[:, bcol:bcol + 1])

                # v : out[s-tile, 8 heads x 64]
                for st_i in range(4):
                    i_blk = 4 * j + st_i
                    ps = pmm.tile([128, 512], F32, name="ps")
                    for k in range(KB):
                        nc.tensor.matmul(
                            ps[:], lhsT=xT[:, k, st_i * 128:(st_i + 1) * 128],
                            rhs=wv_sb[k][:],
                            start=(k == 0), stop=(k == KB - 1))
                    nc.scalar.activation(
                        v_sb[:, i_blk, :, 0:DH],
                        ps[:].rearrange("p (h d) -> p h d", h=GH),
                        AF.Copy)

        # ================= phase C: attention =================
        with tc.tile_pool(name="q2p", bufs=3) as q2p, \
             tc.tile_pool(name="pTp", bufs=3) as pTp, \
             tc.tile_pool(name="rcp", bufs=2) as rcp, \
             tc.tile_pool(name="rbs", bufs=2) as rbs, \
             tc.tile_pool(name="ast", bufs=3) as ast, \
             tc.tile_pool(name="psc", bufs=2, space="PSUM") as psc, \
             tc.tile_pool(name="pat", bufs=3, space="PSUM") as pat, \
             tc.tile_pool(name="pbc", bufs=1, space="PSUM") as pbc:
            for p in range(MT):          # head pair = m-tile
                for j in range(NJ):
                    q2 = q2p.tile([128, 512], F32R, name="q2")
                    nc.sync.dma_start(q2[:], qT_dr[p][:, j * 512:(j + 1) * 512])
                    at_ps = [pat.tile([DH + 1, 512], F32, name="at"),
                             pat.tile([DH + 1, 512], F32, name="at")]
                    nlast = 4 * j + 3
                    for i in range(4 * j + 4):
                        c0 = max(0, i * 128 - j * 512)
                        sc = psc.tile([128, 2, 512], F32, name="sc")
                        for hh in range(2):
                            nc.tensor.matmul(
                                sc[:, hh, c0:],
                                lhsT=kT_sb[p][hh * 64:(hh + 1) * 64,
                                              i * 128:(i + 1) * 128],
                                rhs=q2[hh * 64:(hh + 1) * 64, c0:],
                                start=True, stop=True)
                        pT = pTp.tile([128, 2, 512], F32R, name="pT")
                        nc.scalar.activation(pT[:, :, c0:], sc[:, :, c0:],
                                             AF.Exp, scale=0.125)
                        if i * 128 >= j * 512:  # diagonal block: causal mask
                            for hh in range(2):
                                nc.vector.tensor_tensor(
                                    pT[:, hh, c0:c0 + 128],
                                    pT[:, hh, c0:c0 + 128], tri[:],
                                    op=OP.mult)
                        for hh in range(2):
                            nc.tensor.matmul(
                                at_ps[hh][:, c0:],
                                lhsT=v_sb[:, i, 2 * p + hh, :],
                                rhs=pT[:, hh, c0:],
                                start=(i == 0), stop=(i == nlast))
                    # normalize by ones-column row sum, evac to attnT
                    a_sb = ast.tile([128, 512], F32R, name="a_sb")
                    for hh in range(2):
                        rc = rcp.tile([1, 512], F32R, name="rc")
                        with nc.allow_low_precision("f32r is ~tf32; fine"):
                            nc.vector.reciprocal(rc[:], at_ps[hh][DH:DH + 1, :])
                        rb_ps = pbc.tile([64, 512], F32, name="rb")
                        nc.tensor.matmul(rb_ps[:], lhsT=ones64[:], rhs=rc[:],
                                         start=True, stop=True)
                        rb = rbs.tile([64, 512], F32, name="rbsb")
                        nc.vector.tensor_copy(rb[:], rb_ps[:])
                        nc.vector.tensor_tensor(
                            a_sb[hh * 64:(hh + 1) * 64, :],
                            at_ps[hh][0:DH, :], rb[:], op=OP.mult)
                    nc.sync.dma_start(aT_dr[p][:, j * 512:(j + 1) * 512],
                                      a_sb[:])

        # ================= phase D: output projection =================
        with tc.tile_pool(name="atp", bufs=1) as atp, \
             tc.tile_pool(name="ost", bufs=3) as ost, \
             tc.tile_pool(name="pout", bufs=3, space="PSUM") as pout:
            at_sb = []
            for m in range(MT):
                t = atp.tile([128, S], F32R, name=f"at{m}")
                nc.sync.dma_start(t[:], aT_dr[m][:, :])
                at_sb.append(t)
            for mi in range(ST):
                for n in range(2):
                    ps = pout.tile([128, 512], F32, name="po")
                    for k4 in range(4):
                        nc.tensor.matmul(
                            ps[:],
                            lhsT=at_sb[k4][:, mi * 128:(mi + 1) * 128],
                            rhs=wp_sb[k4 * 2 + n][:],
                            start=(k4 == 0), stop=(k4 == 3))
                    o_sb = ost.tile([128, 512], F32, name="o_sb")
                    nc.scalar.copy(o_sb[:], ps[:])
                    nc.sync.dma_start(
                        out.ap()[mi * 128:(mi + 1) * 128,
                                 n * 512:(n + 1) * 512], o_sb[:])


_NC = None


def _get_module():
    global _NC
    if _NC is None:
        _NC = build_module()
    return _NC


def make_in_maps(hidden_states, c_attn_w, c_attn_b, c_proj_w):
    in_maps = []
    for c in range(NCORES):
        b, g = c // 2, c % 2
        cols = slice(g * GD, (g + 1) * GD)
        bq = np.ascontiguousarray(
            c_attn_b[g * GD:(g + 1) * GD].reshape(MT, 128).T)
        bk = np.ascontiguousarray(
            c_attn_b[D + g * GD:D + (g + 1) * GD].reshape(MT, 128).T)
        in_maps.append({
            "x": np.ascontiguousarray(hidden_states[b]),
            "wq": np.ascontiguousarray(c_attn_w[:, cols]),
            "wk": np.ascontiguousarray(c_attn_w[:, D + g * GD:D + (g + 1) * GD]),
            "wv": np.ascontiguousarray(
                c_attn_w[:, 2 * D + g * GD:2 * D + (g + 1) * GD]),
            "wp": np.ascontiguousarray(c_proj_w[g * GD:(g + 1) * GD, :]),
            "bqk": np.concatenate([bq, bk], axis=1).astype(np.float32),
        })
    return in_maps


def kernel(hidden_states, c_attn_w, c_attn_b, c_proj_w, c_proj_b, _trace=False):
    hidden_states = np.asarray(hidden_states, dtype=np.float32)
    c_attn_w = np.asarray(c_attn_w, dtype=np.float32)
    c_attn_b = np.asarray(c_attn_b, dtype=np.float32)
    c_proj_w = np.asarray(c_proj_w, dtype=np.float32)
    c_proj_b = np.asarray(c_proj_b, dtype=np.float32)

    nc = _get_module()
    in_maps = make_in_maps(hidden_states, c_attn_w, c_attn_b, c_proj_w)
    res = run_bass_kernel_spmd(nc, in_maps, list(range(NCORES)), trace=_trace)

    # v-bias is folded here: attn rows sum to 1, so +b_v passes through the
    # attention average and lands as b_v @ c_proj_w on the output.
    bias_eff = c_proj_b + c_attn_b[2 * D:3 * D] @ c_proj_w
    outp = np.empty((B, S, D), dtype=np.float32)
    for b in range(B):
        outp[b] = (res.results[2 * b]["out"] + res.results[2 * b + 1]["out"]
                   + bias_eff[None, :])
    if _trace:
        return outp, res
    return outp


# revision 9
# speedup vs baseline: 1.1032x; 1.1032x over previous
"""GPT2 attention (B=4, S=2048, D=1024, H=16) on 8 trn2 cores.

Sharding: data-parallel over batch (4) x tensor-parallel over heads (2 groups
of 8). Core c handles batch c//2, head group c%2. Each core computes its
partial output projection (row-split c_proj); the host sums the two partials
per batch and adds the (host-folded) biases.

Per-core kernel (all matmuls in float32r ~ tf32 precision, fp32 accumulate):
  B: x^T via PE transposes -> QKV projections (q^T,k^T feature-major; v
     natural with a ones column appended for softmax row sums)
  C: causal attention, transposed scores: scoresT[sk,sq] = k^T.T @ q^T,
     p^T = exp(scoresT/8) (no max-subtract; |scores| is small), diagonal
     blocks masked by an upper-triangular 0/1 multiply, attn^T accumulated
     in PSUM over sk blocks via lhsT=[v|1]; normalization by the ones-column
     row sum (reciprocal + K=1 broadcast matmul).
  D: out_partial = attnT.T @ c_proj_w (row slice).
"""
import sys

sys.path.insert(0, "/opt/trn_rl_repo")

import numpy as np
from contextlib import ExitStack

import concourse.bass as bass
import concourse.bacc as bacc
import concourse.mybir as mybir
import concourse.tile as tile
from concourse.masks import make_identity, make_upper_triangular
from concourse.bass_utils import run_bass_kernel_spmd

F32 = mybir.dt.float32
F32R = mybir.dt.float32r
AF = mybir.ActivationFunctionType
OP = mybir.AluOpType

B, S, D, H = 4, 2048, 1024, 16
DH = 64            # head dim
NCORES = 8
GH = 8             # heads per core
GD = GH * DH       # 512 feature cols per core
ST = S // 128      # 16 s-tiles
KB = D // 128      # 8 contraction blocks
NJ = S // 512      # 4 sq chunks
MT = GD // 128     # 4 m-tiles (= head pairs)


def build_module(repeat=1):
    nc = bacc.Bacc(None, target_bir_lowering=False, debug=False)

    x = nc.declare_dram_parameter("x", [S, D], F32, isOutput=False)
    wq = nc.declare_dram_parameter("wq", [D, GD], F32, isOutput=False)
    wk = nc.declare_dram_parameter("wk", [D, GD], F32, isOutput=False)
    wv = nc.declare_dram_parameter("wv", [D, GD], F32, isOutput=False)
    wp = nc.declare_dram_parameter("wp", [GD, D], F32, isOutput=False)
    bqk = nc.declare_dram_parameter("bqk", [128, 2 * MT], F32, isOutput=False)
    out = nc.declare_dram_parameter("out", [S, D], F32, isOutput=True)

    with tile.TileContext(nc) as tc, ExitStack() as rctx:
        if repeat > 1:
            rctx.enter_context(tc.For_i(0, repeat, 1))
        _build_body(nc, tc, x, wq, wk, wv, wp, bqk, out)
    nc.compile()
    return nc


def _build_body(nc, tc, x, wq, wk, wv, wp, bqk, out):
    with ExitStack() as ctx:
        const = ctx.enter_context(tc.tile_pool(name="const", bufs=1))
        wpool = ctx.enter_context(tc.tile_pool(name="wpool", bufs=1))
        wppool = ctx.enter_context(tc.tile_pool(name="wppool", bufs=1))
        resid = ctx.enter_context(tc.tile_pool(name="resid", bufs=1))
        dram = ctx.enter_context(tc.tile_pool(name="dram", bufs=1, space="DRAM"))
        xctx = ExitStack()
        xin = xctx.enter_context(tc.tile_pool(name="xin", bufs=6))

        # ---- x loads for j=0 first, so PE transposes can start early ----
        xtiles = {}
        for si in range(4):
            xt = xin.tile([128, D], F32, name="xt")
            nc.sync.dma_start(xt[:], x.ap()[si * 128:(si + 1) * 128, :])
            xtiles[si] = xt

        # ---- constants ----
        ident = const.tile([128, 128], F32)
        make_identity(nc, ident[:])
        tri = const.tile([128, 128], F32)  # 1 where col >= row else 0
        make_upper_triangular(nc, tri[:], val=1.0, diag=True)
        ones64_f = const.tile([1, 64], F32)
        nc.gpsimd.memset(ones64_f[:], 1.0)
        ones64 = const.tile([1, 64], F32R)
        nc.vector.tensor_copy(ones64[:], ones64_f[:])
        ones_v = const.tile([128, ST * GH], F32)
        nc.gpsimd.memset(ones_v[:], 1.0)
        bqk_sb = const.tile([128, 2 * MT], F32)
        nc.sync.dma_start(bqk_sb[:], bqk.ap())

        # ---- weights: load f32, convert to f32r ----
        wq_sb = [wpool.tile([128, GD], F32R, name=f"wq{k}") for k in range(KB)]
        wk_sb = [wpool.tile([128, GD], F32R, name=f"wk{k}") for k in range(KB)]
        wv_sb = [wpool.tile([128, GD], F32R, name=f"wv{k}") for k in range(KB)]
        wp_sb = [wppool.tile([128, 512], F32R, name=f"wp{i}") for i in range(8)]
        with tc.tile_pool(name="wstage", bufs=3) as wstage:
            for k in range(KB):
                for name, dr, sb in (("q", wq, wq_sb), ("k", wk, wk_sb),
                                     ("v", wv, wv_sb)):
                    st_ = wstage.tile([128, GD], F32, name="wst")
                    nc.sync.dma_start(st_[:], dr.ap()[k * 128:(k + 1) * 128, :])
                    nc.vector.tensor_copy(sb[k][:], st_[:])
            for k4 in range(4):
                for n in range(2):
                    st_ = wstage.tile([128, 512], F32, name="wst2")
                    nc.sync.dma_start(
                        st_[:], wp.ap()[k4 * 128:(k4 + 1) * 128,
                                        n * 512:(n + 1) * 512])
                    nc.vector.tensor_copy(wp_sb[k4 * 2 + n][:], st_[:])

        # ---- residents ----
        kT_sb = [resid.tile([128, S], F32R, name=f"kT{m}") for m in range(MT)]
        # v with ones column: [128 part = s-within-block, block i, head, 65]
        v_sb = resid.tile([128, ST, GH, DH + 1], F32R)
        nc.vector.tensor_copy(
            v_sb[:, :, :, DH],
            ones_v[:].rearrange("p (a b) -> p a b", a=ST))

        qT_dr = [dram.tile([128, S], F32R, name=f"qTd{m}") for m in range(MT)]

        # ================= phase B: transposes + QKV =================
        with tc.tile_pool(name="xTp", bufs=2) as xTp, \
             tc.tile_pool(name="qst", bufs=3) as qst, \
             tc.tile_pool(name="ptr", bufs=2, space="PSUM") as ptr, \
             tc.tile_pool(name="pmm", bufs=3, space="PSUM") as pmm:
            for j in range(NJ):
                xts = []
                for st_i in range(4):
                    si = 4 * j + st_i
                    if si in xtiles:
                        xts.append(xtiles.pop(si))
                    else:
                        xt = xin.tile([128, D], F32, name="xt")
                        nc.sync.dma_start(
                            xt[:], x.ap()[si * 128:(si + 1) * 128, :])
                        xts.append(xt)
                xT = xTp.tile([128, KB, 512], F32R, name="xT")
                for k in range(KB):
                    pt = ptr.tile([128, 4, 128], F32, name="pt")
                    for st_i in range(4):
                        nc.tensor.matmul(
                            pt[:, st_i, :],
                            lhsT=xts[st_i][:, k * 128:(k + 1) * 128],
                            rhs=ident[:], is_transpose=True,
                            start=(st_i == 0), stop=(st_i == 3))
                    nc.vector.tensor_copy(xT[:, k, :], pt[:, :, :])

                # q^T, k^T : out[d-col, s-chunk]; bias added per partition
                for m in range(MT):
                    for which, wsb, bcol in ((0, wq_sb, m), (1, wk_sb, MT + m)):
                        ps = pmm.tile([128, 512], F32, name="ps")
                        for k in range(KB):
                            nc.tensor.matmul(
                                ps[:], lhsT=wsb[k][:, m * 128:(m + 1) * 128],
                                rhs=xT[:, k, :],
                                start=(k == 0), stop=(k == KB - 1))
                        if which == 0:
                            qs = qst.tile([128, 512], F32R, name="qs")
                            nc.vector.tensor_scalar_add(
                                qs[:], ps[:], bqk_sb[:, bcol:bcol + 1])
                            nc.sync.dma_start(
                                qT_dr[m][:, j * 512:(j + 1) * 512], qs[:])
                        else:
                            nc.vector.tensor_scalar_add(
                                kT_sb[m][:, j * 512:(j + 1) * 512], ps[:],
                                bqk_sb[:, bcol:bcol + 1])

                # v : out[s-tile, 8 heads x 64]
                for st_i in range(4):
                    i_blk = 4 * j + st_i
                    ps = pmm.tile([128, 512], F32, name="ps")
                    for k in range(KB):
                        nc.tensor.matmul(
                            ps[:], lhsT=xT[:, k, st_i * 128:(st_i + 1) * 128],
                            rhs=wv_sb[k][:],
                            start=(k == 0), stop=(k == KB - 1))
                    nc.vector.tensor_copy(
                        v_sb[:, i_blk, :, 0:DH],
                        ps[:].rearrange("p (h d) -> p h d", h=GH))
        xctx.close()

        # ========== phase C+D: attention (j outer) + projection per j ======
        with tc.tile_pool(name="q2p", bufs=3) as q2p, \
             tc.tile_pool(name="pTp", bufs=3) as pTp, \
             tc.tile_pool(name="rcp", bufs=2) as rcp, \
             tc.tile_pool(name="rbs", bufs=2) as rbs, \
             tc.tile_pool(name="ast", bufs=8) as ast, \
             tc.tile_pool(name="ost", bufs=3) as ost, \
             tc.tile_pool(name="psc", bufs=2, space="PSUM") as psc, \
             tc.tile_pool(name="pat", bufs=2, space="PSUM") as pat, \
             tc.tile_pool(name="psm", bufs=2, space="PSUM") as psm:
            for j in range(NJ):
                a_tiles = []
                for p in range(MT):          # head pair = m-tile
                    q2 = q2p.tile([128, 512], F32R, name="q2")
                    nc.sync.dma_start(q2[:], qT_dr[p][:, j * 512:(j + 1) * 512])
                    at_ps = [pat.tile([DH + 1, 512], F32, name="at"),
                             pat.tile([DH + 1, 512], F32, name="at")]
                    nlast = 4 * j + 3
                    for i in range(4 * j + 4):
                        c0 = max(0, i * 128 - j * 512)
                        sc = psc.tile([128, 2, 512], F32, name="sc")
                        for hh in range(2):
                            nc.tensor.matmul(
                                sc[:, hh, c0:],
                                lhsT=kT_sb[p][hh * 64:(hh + 1) * 64,
                                              i * 128:(i + 1) * 128],
                                rhs=q2[hh * 64:(hh + 1) * 64, c0:],
                                start=True, stop=True)
                        pT = pTp.tile([128, 2, 512], F32R, name="pT")
                        nc.scalar.activation(pT[:, :, c0:], sc[:, :, c0:],
                                             AF.Exp, scale=0.125)
                        if i * 128 >= j * 512:  # diagonal block: causal mask
                            for hh in range(2):
                                nc.vector.tensor_tensor(
                                    pT[:, hh, c0:c0 + 128],
                                    pT[:, hh, c0:c0 + 128], tri[:],
                                    op=OP.mult)
                        for hh in range(2):
                            nc.tensor.matmul(
                                at_ps[hh][:, c0:],
                                lhsT=v_sb[:, i, 2 * p + hh, :],
                                rhs=pT[:, hh, c0:],
                                start=(i == 0), stop=(i == nlast))
                    # normalize by ones-column row sum, evac to attnT (SBUF)
                    a_sb = ast.tile([128, 512], F32R, name="a_sb")
                    for hh in range(2):
                        rc = rcp.tile([1, 512], F32R, name="rc")
                        with nc.allow_low_precision("f32r is ~tf32; fine"):
                            nc.vector.reciprocal(rc[:], at_ps[hh][DH:DH + 1, :])
                        rb_ps = psm.tile([128, 512], F32, name="psm")
                        nc.tensor.matmul(rb_ps[0:64, :], lhsT=ones64[:],
                                         rhs=rc[:], start=True, stop=True)
                        rb = rbs.tile([64, 512], F32, name="rbsb")
                        nc.vector.tensor_copy(rb[:], rb_ps[0:64, :])
                        nc.vector.tensor_tensor(
                            a_sb[hh * 64:(hh + 1) * 64, :],
                            at_ps[hh][0:DH, :], rb[:], op=OP.mult)
                    a_tiles.append(a_sb)

                # projection for this j: out rows j*512 .. j*512+511
                for mi4 in range(4):
                    mi = 4 * j + mi4
                    for n in range(2):
                        ps = psm.tile([128, 512], F32, name="psm")
                        for k4 in range(4):
                            nc.tensor.matmul(
                                ps[:],
                                lhsT=a_tiles[k4][:, mi4 * 128:(mi4 + 1) * 128],
                                rhs=wp_sb[k4 * 2 + n][:],
                                start=(k4 == 0), stop=(k4 == 3))
                        o_sb = ost.tile([128, 512], F32, name="o_sb")
                        nc.vector.tensor_copy(o_sb[:], ps[:])
                        nc.sync.dma_start(
                            out.ap()[mi * 128:(mi + 1) * 128,
                                     n * 512:(n + 1) * 512], o_sb[:])


_NC = None


def _get_module():
    global _NC
    if _NC is None:
        _NC = build_module()
    return _NC


def make_in_maps(hidden_states, c_attn_w, c_attn_b, c_proj_w):
    in_maps = []
    for c in range(NCORES):
        b, g = c // 2, c % 2
        cols = slice(g * GD, (g + 1) * GD)
        bq = np.ascontiguousarray(
            c_attn_b[g * GD:(g + 1) * GD].reshape(MT, 128).T)
        bk = np.ascontiguousarray(
            c_attn_b[D + g * GD:D + (g + 1) * GD].reshape(MT, 128).T)
        in_maps.append({
            "x": np.ascontiguousarray(hidden_states[b]),
            "wq": np.ascontiguousarray(c_attn_w[:, cols]),
            "wk": np.ascontiguousarray(c_attn_w[:, D + g * GD:D + (g + 1) * GD]),
            "wv": np.ascontiguousarray(
                c_attn_w[:, 2 * D + g * GD:2 * D + (g + 1) * GD]),
            "wp": np.ascontiguousarray(c_proj_w[g * GD:(g + 1) * GD, :]),
            "bqk": np.concatenate([bq, bk], axis=1).astype(np.float32),
        })
    return in_maps


def kernel(hidden_states, c_attn_w, c_attn_b, c_proj_w, c_proj_b, _trace=False):
    hidden_states = np.asarray(hidden_states, dtype=np.float32)
    c_attn_w = np.asarray(c_attn_w, dtype=np.float32)
    c_attn_b = np.asarray(c_attn_b, dtype=np.float32)
    c_proj_w = np.asarray(c_proj_w, dtype=np.float32)
    c_proj_b = np.asarray(c_proj_b, dtype=np.float32)

    nc = _get_module()
    in_maps = make_in_maps(hidden_states, c_attn_w, c_attn_b, c_proj_w)
    res = run_bass_kernel_spmd(nc, in_maps, list(range(NCORES)), trace=_trace)

    # v-bias is folded here: attn rows sum to 1, so +b_v passes through the
    # attention average and lands as b_v @ c_proj_w on the output.
    bias_eff = c_proj_b + c_attn_b[2 * D:3 * D] @ c_proj_w
    outp = np.empty((B, S, D), dtype=np.float32)
    for b in range(B):
        outp[b] = (res.results[2 * b]["out"] + res.results[2 * b + 1]["out"]
                   + bias_eff[None, :])
    if _trace:
        return outp, res
    return outp


# revision 10
# speedup vs baseline: 1.2895x; 1.1688x over previous
"""GPT2 attention (B=4, S=2048, D=1024, H=16) on 8 trn2 cores.

Sharding: data-parallel over batch (4) x tensor-parallel over heads (2 groups
of 8). Core c handles batch c//2, head group c%2. Each core computes its
partial output projection (row-split c_proj); the host sums the two partials
per batch and adds the (host-folded) biases.

Per-core kernel (all matmuls in float32r ~ tf32 precision, fp32 accumulate):
  B: x^T via PE transposes -> QKV projections (q^T,k^T feature-major; v
     natural with a ones column appended for softmax row sums)
  C: causal attention, transposed scores: scoresT[sk,sq] = k^T.T @ q^T,
     p^T = exp(scoresT/8) (no max-subtract; |scores| is small), diagonal
     blocks masked by an upper-triangular 0/1 multiply, attn^T accumulated
     in PSUM over sk blocks via lhsT=[v|1]; normalization by the ones-column
     row sum (reciprocal + K=1 broadcast matmul).
  D: out_partial = attnT.T @ c_proj_w (row slice).
"""
import sys

sys.path.insert(0, "/opt/trn_rl_repo")

import numpy as np
from contextlib import ExitStack

import concourse.bass as bass
import concourse.bacc as bacc
import concourse.mybir as mybir
import concourse.tile as tile
from concourse.masks import make_identity, make_upper_triangular
from concourse.bass_utils import run_bass_kernel_spmd

F32 = mybir.dt.float32
F32R = mybir.dt.float32r
AF = mybir.ActivationFunctionType
OP = mybir.AluOpType

B, S, D, H = 4, 2048, 1024, 16
DH = 64            # head dim
NCORES = 8
GH = 8             # heads per core
GD = GH * DH       # 512 feature cols per core
ST = S // 128      # 16 s-tiles
KB = D // 128      # 8 contraction blocks
NJ = S // 512      # 4 sq chunks
MT = GD // 128     # 4 m-tiles (= head pairs)


def build_module(repeat=1):
    nc = bacc.Bacc(None, target_bir_lowering=False, debug=False)

    x = nc.declare_dram_parameter("x", [S, D], F32R, isOutput=False)
    wq = nc.declare_dram_parameter("wq", [D, GD], F32R, isOutput=False)
    wk = nc.declare_dram_parameter("wk", [D, GD], F32R, isOutput=False)
    wv = nc.declare_dram_parameter("wv", [D, GD], F32R, isOutput=False)
    wp = nc.declare_dram_parameter("wp", [GD, D], F32R, isOutput=False)
    bqk = nc.declare_dram_parameter("bqk", [128, 2 * MT], F32, isOutput=False)
    out = nc.declare_dram_parameter("out", [S, D], F32, isOutput=True)

    with tile.TileContext(nc) as tc, ExitStack() as rctx:
        if repeat > 1:
            rctx.enter_context(tc.For_i(0, repeat, 1))
        _build_body(nc, tc, x, wq, wk, wv, wp, bqk, out)
    nc.compile()
    return nc


def _build_body(nc, tc, x, wq, wk, wv, wp, bqk, out):
    with ExitStack() as ctx:
        const = ctx.enter_context(tc.tile_pool(name="const", bufs=1))
        wpool = ctx.enter_context(tc.tile_pool(name="wpool", bufs=1))
        wppool = ctx.enter_context(tc.tile_pool(name="wppool", bufs=1))
        resid = ctx.enter_context(tc.tile_pool(name="resid", bufs=1))
        dram = ctx.enter_context(tc.tile_pool(name="dram", bufs=1, space="DRAM"))
        xctx = ExitStack()
        xin = xctx.enter_context(tc.tile_pool(name="xin", bufs=6))

        # ---- x loads for j=0 first, so PE transposes can start early ----
        xtiles = {}
        for si in range(4):
            xt = xin.tile([128, D], F32R, name="xt")
            nc.sync.dma_start(xt[:], x.ap()[si * 128:(si + 1) * 128, :])
            xtiles[si] = xt

        # ---- constants ----
        ident_f = const.tile([128, 128], F32)
        make_identity(nc, ident_f[:])
        ident = const.tile([128, 128], F32R)
        nc.vector.tensor_copy(ident[:], ident_f[:])
        tri_f = const.tile([128, 128], F32)  # 1 where col >= row else 0
        make_upper_triangular(nc, tri_f[:], val=1.0, diag=True)
        tri = const.tile([128, 128], F32R)
        nc.vector.tensor_copy(tri[:], tri_f[:])
        ones64_f = const.tile([1, 64], F32)
        nc.gpsimd.memset(ones64_f[:], 1.0)
        ones64 = const.tile([1, 64], F32R)
        nc.vector.tensor_copy(ones64[:], ones64_f[:])
        ones_v = const.tile([128, ST * GH], F32)
        nc.gpsimd.memset(ones_v[:], 1.0)
        bqk_sb = const.tile([128, 2 * MT], F32)
        nc.sync.dma_start(bqk_sb[:], bqk.ap())

        # ---- weights: load f32, convert to f32r ----
        wq_sb = [wpool.tile([128, GD], F32R, name=f"wq{k}") for k in range(KB)]
        wk_sb = [wpool.tile([128, GD], F32R, name=f"wk{k}") for k in range(KB)]
        wv_sb = [wpool.tile([128, GD], F32R, name=f"wv{k}") for k in range(KB)]
        wp_sb = [wppool.tile([128, 512], F32R, name=f"wp{i}") for i in range(8)]
        for k in range(KB):
            for dr, sb in ((wq, wq_sb), (wk, wk_sb), (wv, wv_sb)):
                nc.gpsimd.dma_start(sb[k][:], dr.ap()[k * 128:(k + 1) * 128, :])
        for k4 in range(4):
            for n in range(2):
                nc.gpsimd.dma_start(
                    wp_sb[k4 * 2 + n][:],
                    wp.ap()[k4 * 128:(k4 + 1) * 128, n * 512:(n + 1) * 512])

        # ---- residents ----
        kT_sb = [resid.tile([128, S], F32R, name=f"kT{m}") for m in range(MT)]
        # v with ones column: [128 part = s-within-block, block i, head, 65]
        v_sb = resid.tile([128, ST, GH, DH + 1], F32R)
        nc.vector.tensor_copy(
            v_sb[:, :, :, DH],
            ones_v[:].rearrange("p (a b) -> p a b", a=ST))

        qT_dr = [dram.tile([128, S], F32R, name=f"qTd{m}") for m in range(MT)]

        # ================= phase B: transposes + QKV =================
        with tc.tile_pool(name="xTp", bufs=2) as xTp, \
             tc.tile_pool(name="qst", bufs=3) as qst, \
             tc.tile_pool(name="ptr", bufs=2, space="PSUM") as ptr, \
             tc.tile_pool(name="pmm", bufs=3, space="PSUM") as pmm:
            for j in range(NJ):
                xts = []
                for st_i in range(4):
                    si = 4 * j + st_i
                    if si in xtiles:
                        xts.append(xtiles.pop(si))
                    else:
                        xt = xin.tile([128, D], F32R, name="xt")
                        nc.sync.dma_start(
                            xt[:], x.ap()[si * 128:(si + 1) * 128, :])
                        xts.append(xt)
                xT = xTp.tile([128, KB, 512], F32R, name="xT")
                for k in range(KB):
                    pt = ptr.tile([128, 4, 128], F32R, name="pt")
                    for st_i in range(4):
                        nc.tensor.matmul(
                            pt[:, st_i, :],
                            lhsT=xts[st_i][:, k * 128:(k + 1) * 128],
                            rhs=ident[:], is_transpose=True,
                            start=(st_i == 0), stop=(st_i == 3))
                    nc.vector.tensor_copy(xT[:, k, :], pt[:, :, :])

                # q^T, k^T : out[d-col, s-chunk]; bias added per partition
                for m in range(MT):
                    for which, wsb, bcol in ((0, wq_sb, m), (1, wk_sb, MT + m)):
                        ps = pmm.tile([128, 512], F32, name="ps")
                        for k in range(KB):
                            nc.tensor.matmul(
                                ps[:], lhsT=wsb[k][:, m * 128:(m + 1) * 128],
                                rhs=xT[:, k, :],
                                start=(k == 0), stop=(k == KB - 1))
                        if which == 0:
                            qs = qst.tile([128, 512], F32R, name="qs")
                            nc.vector.tensor_scalar_add(
                                qs[:], ps[:], bqk_sb[:, bcol:bcol + 1])
                            nc.sync.dma_start(
                                qT_dr[m][:, j * 512:(j + 1) * 512], qs[:])
                        else:
                            nc.vector.tensor_scalar_add(
                                kT_sb[m][:, j * 512:(j + 1) * 512], ps[:],
                                bqk_sb[:, bcol:bcol + 1])

                # v : out[s-tile, 8 heads x 64]
                for st_i in range(4):
                    i_blk = 4 * j + st_i
                    ps = pmm.tile([128, 512], F32, name="ps")
                    for k in range(KB):
                        nc.tensor.matmul(
                            ps[:], lhsT=xT[:, k, st_i * 128:(st_i + 1) * 128],
                            rhs=wv_sb[k][:],
                            start=(k == 0), stop=(k == KB - 1))
                    nc.vector.tensor_copy(
                        v_sb[:, i_blk, :, 0:DH],
                        ps[:].rearrange("p (h d) -> p h d", h=GH))
        xctx.close()

        # ========== phase C+D: attention (j outer) + projection per j ======
        with tc.tile_pool(name="q2p", bufs=3) as q2p, \
             tc.tile_pool(name="pTp", bufs=3) as pTp, \
             tc.tile_pool(name="rcp", bufs=2) as rcp, \
             tc.tile_pool(name="rbs", bufs=2) as rbs, \
             tc.tile_pool(name="ast", bufs=8) as ast, \
             tc.tile_pool(name="ost", bufs=3) as ost, \
             tc.tile_pool(name="psc", bufs=2, space="PSUM") as psc, \
             tc.tile_pool(name="pat", bufs=2, space="PSUM") as pat, \
             tc.tile_pool(name="psm", bufs=2, space="PSUM") as psm:
            for j in range(NJ):
                a_tiles = []
                for p in range(MT):          # head pair = m-tile
                    q2 = q2p.tile([128, 512], F32R, name="q2")
                    nc.sync.dma_start(q2[:], qT_dr[p][:, j * 512:(j + 1) * 512])
                    at_ps = [pat.tile([DH + 1, 512], F32, name="at"),
                             pat.tile([DH + 1, 512], F32, name="at")]
                    nlast = 4 * j + 3
                    for i in range(4 * j + 4):
                        c0 = max(0, i * 128 - j * 512)
                        sc = psc.tile([128, 2, 512], F32, name="sc")
                        for hh in range(2):
                            nc.tensor.matmul(
                                sc[:, hh, c0:],
                                lhsT=kT_sb[p][hh * 64:(hh + 1) * 64,
                                              i * 128:(i + 1) * 128],
                                rhs=q2[hh * 64:(hh + 1) * 64, c0:],
                                start=True, stop=True)
                        pT = pTp.tile([128, 2, 512], F32R, name="pT")
                        nc.scalar.activation(pT[:, :, c0:], sc[:, :, c0:],
                                             AF.Exp, scale=0.125)
                        if i * 128 >= j * 512:  # diagonal block: causal mask
                            nc.vector.tensor_tensor(
                                pT[:, :, c0:c0 + 128],
                                pT[:, :, c0:c0 + 128],
                                tri[:, None, :].broadcast_to([128, 2, 128]),
                                op=OP.mult)
                        for hh in range(2):
                            nc.tensor.matmul(
                                at_ps[hh][:, c0:],
                                lhsT=v_sb[:, i, 2 * p + hh, :],
                                rhs=pT[:, hh, c0:],
                                start=(i == 0), stop=(i == nlast))
                    # normalize by ones-column row sum, evac to attnT (SBUF)
                    a_sb = ast.tile([128, 512], F32R, name="a_sb")
                    for hh in range(2):
                        rc = rcp.tile([1, 512], F32R, name="rc")
                        with nc.allow_low_precision("f32r is ~tf32; fine"):
                            nc.vector.reciprocal(rc[:], at_ps[hh][DH:DH + 1, :])
                        rb_ps = psm.tile([128, 512], F32, name="psm")
                        nc.tensor.matmul(rb_ps[0:64, :], lhsT=ones64[:],
                                         rhs=rc[:], start=True, stop=True)
                        rb = rbs.tile([64, 512], F32, name="rbsb")
                        nc.vector.tensor_copy(rb[:], rb_ps[0:64, :])
                        nc.vector.tensor_tensor(
                            a_sb[hh * 64:(hh + 1) * 64, :],
                            at_ps[hh][0:DH, :], rb[:], op=OP.mult)
                    a_tiles.append(a_sb)

                # projection for this j: out rows j*512 .. j*512+511
                for mi4 in range(4):
                    mi = 4 * j + mi4
                    for n in range(2):
                        ps = psm.tile([128, 512], F32, name="psm")
                        for k4 in range(4):
                            nc.tensor.matmul(
                                ps[:],
                                lhsT=a_tiles[k4][:, mi4 * 128:(mi4 + 1) * 128],
                                rhs=wp_sb[k4 * 2 + n][:],
                                start=(k4 == 0), stop=(k4 == 3))
                        o_sb = ost.tile([128, 512], F32, name="o_sb")
                        nc.vector.tensor_copy(o_sb[:], ps[:])
                        nc.sync.dma_start(
                            out.ap()[mi * 128:(mi + 1) * 128,
                                     n * 512:(n + 1) * 512], o_sb[:])


_NC = None


def _get_module():
    global _NC
    if _NC is None:
        _NC = build_module()
    return _NC


def make_in_maps(hidden_states, c_attn_w, c_attn_b, c_proj_w):
    in_maps = []
    for c in range(NCORES):
        b, g = c // 2, c % 2
        cols = slice(g * GD, (g + 1) * GD)
        bq = np.ascontiguousarray(
            c_attn_b[g * GD:(g + 1) * GD].reshape(MT, 128).T)
        bk = np.ascontiguousarray(
            c_attn_b[D + g * GD:D + (g + 1) * GD].reshape(MT, 128).T)
        in_maps.append({
            "x": np.ascontiguousarray(hidden_states[b]),
            "wq": np.ascontiguousarray(c_attn_w[:, cols]),
            "wk": np.ascontiguousarray(c_attn_w[:, D + g * GD:D + (g + 1) * GD]),
            "wv": np.ascontiguousarray(
                c_attn_w[:, 2 * D + g * GD:2 * D + (g + 1) * GD]),
            "wp": np.ascontiguousarray(c_proj_w[g * GD:(g + 1) * GD, :]),
            "bqk": np.concatenate([bq, bk], axis=1).astype(np.float32),
        })
    return in_maps


def kernel(hidden_states, c_attn_w, c_attn_b, c_proj_w, c_proj_b, _trace=False):
    hidden_states = np.asarray(hidden_states, dtype=np.float32)
    c_attn_w = np.asarray(c_attn_w, dtype=np.float32)
    c_attn_b = np.asarray(c_attn_b, dtype=np.float32)
    c_proj_w = np.asarray(c_proj_w, dtype=np.float32)
    c_proj_b = np.asarray(c_proj_b, dtype=np.float32)

    nc = _get_module()
    in_maps = make_in_maps(hidden_states, c_attn_w, c_attn_b, c_proj_w)
    res = run_bass_kernel_spmd(nc, in_maps, list(range(NCORES)), trace=_trace)

    # v-bias is folded here: attn rows sum to 1, so +b_v passes through the
    # attention average and lands as b_v @ c_proj_w on the output.
    bias_eff = c_proj_b + c_attn_b[2 * D:3 * D] @ c_proj_w
    outp = np.empty((B, S, D), dtype=np.float32)
    for b in range(B):
        outp[b] = (res.results[2 * b]["out"] + res.results[2 * b + 1]["out"]
                   + bias_eff[None, :])
    if _trace:
        return outp, res
    return outp
